# revision 57
# baseline (speedup 1.0000x reference)
"""Trainium2 Bass kernel for a single pre-norm transformer block.

Reference math (B=4, T=2048, C=512, H=8, D=64, fp32):
    h  = LN(x; g1, b1) ; q,k,v = h @ Wq/Wk/Wv (per head)
    wei = softmax_over_QUERY_axis( causal_mask(q k^T / sqrt(C)) )
    x2  = x + concat_heads(wei @ v) @ Wo + bo
    out = x2 + relu(LN(x2; g2, b2) @ W1 + b1) @ W2 + b2

Sharding over 8 NeuronCores: pairs of cores per batch element
(core = 2*b + r). Each core computes LN1 + QKV for its batch,
attention for its 4 heads (h = 4r..4r+3), and the partial output
projection (contracting only its heads' features). A pairwise
ReduceScatter sums the partial projections and hands each core its
half of the tokens; the FFN is token-parallel (1024 tokens/core).

Numerics: W1 and Wo run on fp8e4m3 operands in DoubleRow perf mode
(K=256 per instruction at 0.5 cyc/row), with hi+lo fp8 splits of both
operands (x ~ hi + lo keeps ~7 mantissa bits; 3 cross products) so
the error stays at bf16 level. W2 contracts fp8 relu activations
(single quantization, the dominant error term at ~1.4e-2 rel) against
a hi+lo split of W2. QKV, attention scores and AV stay bf16. The
per-tensor power-of-2 scales are absorbed by the exp's scale operand
and the epilogue multiplies, so no extra passes are spent on
rescaling. Engine budget: ACT is reserved for the softmax exp during
attention (the hard floor at ~73us/core); LN/copy/relu traffic is
spread across DVE/Pool/ACT by phase; Wo + ReduceScatter + LN2 + the
first FFN half are pipelined into head 3's attention tail.
"""

import sys

sys.path.insert(0, "/opt/trn_rl_repo")

import ml_dtypes
import numpy as np

B, T, C, H, D = 4, 2048, 512, 8, 64
EPS = 1e-5
NCORES = 8
TH = T // 2  # tokens per core in the FFN phase
HPC = H // 2  # heads per core
NT = T // 128  # 16 token tiles per batch
NEG = -1e30
NCH = 4  # collective chunks

SH = 32.0  # scale on LN outputs before fp8 (|h| <= ~5.2 -> 166 < 240)
SA = 16.0  # scale on attention outputs before fp8
SR = 16.0  # scale on relu outputs before fp8

E4NP = ml_dtypes.float8_e4m3
FP8W2 = True  # fp8 relu activations against split-fp8 W2 (vs all-bf16 W2)

_CACHE: dict = {}


def _build_program(flags, sim=False):
    from contextlib import ExitStack

    import concourse.bacc as bacc
    import concourse.bass as bass
    import concourse.tile as tile
    from concourse import mybir
    from concourse.masks import make_identity

    has_bqkv, has_bo, has_b2 = flags
    f32 = mybir.dt.float32
    bf16 = mybir.dt.bfloat16
    f8 = mybir.dt.float8e4
    AF = mybir.ActivationFunctionType
    ALU = mybir.AluOpType
    DR = mybir.MatmulPerfMode.DoubleRow

    nc = bacc.Bacc(
        "TRN2", target_bir_lowering=False, debug=False,
        num_devices=1 if sim else NCORES,
    )

    x_e = nc.dram_tensor("x", [T, C], bf16, kind="ExternalInput").ap()
    xh_e = nc.dram_tensor("xh", [TH, C], f32, kind="ExternalInput").ap()
    wq_e = nc.dram_tensor("wq", [C, HPC * D], bf16,
                          kind="ExternalInput").ap()
    wk_e = nc.dram_tensor("wk", [C, HPC * D], bf16,
                          kind="ExternalInput").ap()
    wv_e = nc.dram_tensor("wv", [C, HPC * D], bf16,
                          kind="ExternalInput").ap()
    wo_es = [nc.dram_tensor(f"wo{s}", [128, 2, C], f8,
                            kind="ExternalInput").ap() for s in "hl"]
    w1_es = [nc.dram_tensor(f"w1{s}", [C, 4 * C], f8,
                            kind="ExternalInput").ap() for s in "hl"]
    if FP8W2:
        w2_es = [nc.dram_tensor(f"w2{s}", [4 * C, C], f8,
                                kind="ExternalInput").ap() for s in "hl"]
    else:
        w2_e = nc.dram_tensor("w2", [4 * C, C], bf16,
                              kind="ExternalInput").ap()
    b1r_e = nc.dram_tensor("b1r", [4 * C], f32, kind="ExternalInput").ap()
    b1s_e = nc.dram_tensor("b1s", [4 * C], f32, kind="ExternalInput").ap()
    # scales: col 0 qks=1/(SH^2 cq ck), 1 vsc=1/(SH cv), 2 wos=1/(SA cwo),
    #         3 rsc=SR/(SH c1), 4 w2s=1/(SR c2)
    sc_e = nc.dram_tensor("scales", [128, 8], f32, kind="ExternalInput").ap()
    if has_bqkv:
        bq_e = nc.dram_tensor("bq", [1, HPC * D], f32, kind="ExternalInput").ap()
        bk_e = nc.dram_tensor("bk", [1, HPC * D], f32, kind="ExternalInput").ap()
        bv_e = nc.dram_tensor("bv", [1, HPC * D], f32, kind="ExternalInput").ap()
    if has_bo:
        bo_e = nc.dram_tensor("bo", [C], f32, kind="ExternalInput").ap()
    if has_b2:
        b2_e = nc.dram_tensor("b2", [C], f32, kind="ExternalInput").ap()
    y_e = nc.dram_tensor("y", [TH, C], f32, kind="ExternalOutput").ap()

    cc_in = [nc.dram_tensor(f"cc_in{k}", [T // NCH, C], bf16)
             for k in range(NCH)]
    cc_out = [nc.dram_tensor(f"cc_out{k}", [T // NCH // 2, C], bf16)
              for k in range(NCH)]

    with tile.TileContext(nc) as tc, ExitStack() as ctx:
        psA = ctx.enter_context(tc.tile_pool(name="psA", bufs=2, space="PSUM"))
        psB = ctx.enter_context(tc.tile_pool(name="psB", bufs=2, space="PSUM"))
        consts = ctx.enter_context(tc.tile_pool(name="consts", bufs=1))
        smalls = ctx.enter_context(tc.tile_pool(name="smalls", bufs=2))
        qk_pool = ctx.enter_context(tc.tile_pool(name="qk", bufs=1))
        vpool = ctx.enter_context(tc.tile_pool(name="vp", bufs=1))
        attnp = ctx.enter_context(tc.tile_pool(name="attnp", bufs=1))
        fw = ctx.enter_context(tc.tile_pool(name="fw", bufs=1))
        x2p = ctx.enter_context(tc.tile_pool(name="x2p", bufs=1))
        h2p = ctx.enter_context(tc.tile_pool(name="h2p", bufs=1))

        # ---- constants ----
        ident_b = consts.tile([128, 128], bf16)
        make_identity(nc, ident_b)
        mb_b = consts.tile([128, 128], bf16)
        nc.gpsimd.memset(mb_b[:], 0.0)
        nc.gpsimd.affine_select(
            out=mb_b[:], in_=mb_b[:], compare_op=ALU.is_ge, fill=NEG,
            base=0, pattern=[[1, 128]], channel_multiplier=-1,
        )
        eps_t = consts.tile([128, 1], f32)
        nc.vector.memset(eps_t[:], EPS / (SH * SH))
        b1r_sb = consts.tile([128, 16], f32)
        b1s_sb = consts.tile([128, 16], f32)
        sc_sb = consts.tile([128, 8], f32)
        qks = sc_sb[:, 0:1]
        vsc = sc_sb[:, 1:2]
        wos = sc_sb[:, 2:3]
        rsc = sc_sb[:, 3:4]
        w2s = sc_sb[:, 4:5]
        if has_bqkv:
            ones_sb = consts.tile([1, 512], f32)
            nc.vector.memset(ones_sb[:], 1.0)
            bq_sb = consts.tile([1, HPC * D], f32)
            nc.sync.dma_start(bq_sb[:], bq_e)
            bk_sb = consts.tile([1, HPC * D], f32)
            nc.sync.dma_start(bk_sb[:], bk_e)
            bv_sb = consts.tile([1, HPC * D], f32)
            nc.sync.dma_start(bv_sb[:], bv_e)
        if has_bo:
            bo_sb = consts.tile([128, C], f32)
            bo_b = bo_e[None, :]
            bo_bc = bass.AP(
                tensor=bo_b.tensor, offset=bo_b.offset,
                ap=[[0, 128], bo_b.ap[1]],
            )
            nc.sync.dma_start(bo_sb[:], bo_bc)
        if has_b2:
            b2_sb = consts.tile([128, C], f32)
            b2_b = b2_e[None, :]
            b2_bc = bass.AP(
                tensor=b2_b.tensor, offset=b2_b.offset,
                ap=[[0, 128], b2_b.ap[1]],
            )
            nc.sync.dma_start(b2_sb[:], b2_bc)

        # persistent activations
        qT = qk_pool.tile([128, 2, T], bf16)  # [pair-head d, pair, t]
        kT = qk_pool.tile([128, 2, T], bf16)
        v_sb = vpool.tile([128, NT, HPC * D], bf16)  # [s in tile, tile, hd]
        attn_hi = attnp.tile([128, 2, T], f8)  # [hd in pair, pair, t] * SA
        attn_lo = attnp.tile([128, 2, T], f8)

        # FFN weights + residual stream (DMA emitted inside phase A, after
        # the x loads, so the x tiles win the DMA queue)
        w1_sb = [fw.tile([128, 4, 4 * C], f8, name=f"w1_{s}")
                 for s in range(2)]
        if FP8W2:
            w2_sb = [fw.tile([128, 16, C], f8, name=f"w2_{s}")
                     for s in range(2)]
        else:
            w2_sb = fw.tile([128, 16, C], bf16)
        wo_sb = [fw.tile([128, 2, C], f8, name=f"wo_{s}")
                 for s in range(2)]
        xh_sb = fw.tile([128, 8, C], f32)
        x2 = x2p.tile([128, 8, C], f32)
        h2T = [h2p.tile([128, 4, TH], f8, name=f"h2T_{s}")
               for s in range(2)]

        def layer_norm_tile(xm, hm, dve=False):
            """hm = SH * (xm - mean) * rsqrt(var + eps)."""
            stats = smalls.tile([128, 6], f32, tag="bnst")
            nc.vector.bn_stats(stats[:], xm)
            mv = smalls.tile([128, 2], f32, tag="bnag")
            nc.vector.bn_aggr(mv[:], stats[:])
            rstd = smalls.tile([128, 1], f32, tag="rstd")
            nc.scalar.activation(rstd[:], mv[:, 1:2], AF.Sqrt,
                                 bias=eps_t[:], scale=1.0 / (SH * SH))
            nc.vector.reciprocal(rstd[:], rstd[:])
            if dve:
                nc.vector.tensor_scalar(
                    hm, xm, mv[:, 0:1], rstd[:], ALU.subtract, ALU.mult
                )
                return
            nmr = smalls.tile([128, 1], f32, tag="nmr")
            nc.vector.tensor_scalar(
                nmr[:], mv[:, 0:1], rstd[:], -1.0, ALU.mult, ALU.mult
            )
            nc.scalar.activation(hm, xm, AF.Identity, bias=nmr[:],
                                 scale=rstd[:])

        # ================= Phase A: LN1 + QKV =================
        with ExitStack() as phaseA:
            wq_pool = phaseA.enter_context(tc.tile_pool(name="wqkv", bufs=1))
            wq_sb = wq_pool.tile([128, 4, HPC * D], bf16)
            wk_sb = wq_pool.tile([128, 4, HPC * D], bf16)
            wv_sb = wq_pool.tile([128, 4, HPC * D], bf16)

            hT_pool = phaseA.enter_context(tc.tile_pool(name="hT", bufs=1))
            hT = hT_pool.tile([128, 4, T], bf16)

            xpool = phaseA.enter_context(tc.tile_pool(name="xp", bufs=1))
            hpool = phaseA.enter_context(tc.tile_pool(name="hn", bufs=1))
            x_sb = xpool.tile([128, NT, C], bf16)
            x_r = x_e.rearrange("(n p) c -> p n c", p=128)
            for xc in range(16):
                nc.sync.dma_start(
                    x_sb[:, xc:xc + 1, :], x_r[:, xc:xc + 1, :],
                )
            nc.sync.dma_start(
                wq_sb[:], wq_e.rearrange("(o p) d -> p o d", p=128))
            nc.sync.dma_start(
                wk_sb[:], wk_e.rearrange("(o p) d -> p o d", p=128))
            nc.sync.dma_start(
                wv_sb[:], wv_e.rearrange("(o p) d -> p o d", p=128))
            for s in range(2):
                nc.sync.dma_start(
                    w1_sb[s][:], w1_es[s].rearrange("(o p) n -> p o n", p=128))
            for s in range(2):
                nc.sync.dma_start(wo_sb[s][:], wo_es[s])
            if FP8W2:
                for s in range(2):
                    nc.sync.dma_start(
                        w2_sb[s][:],
                        w2_es[s].rearrange("(o p) c -> p o c", p=128))
            else:
                nc.sync.dma_start(
                    w2_sb[:], w2_e.rearrange("(o p) c -> p o c", p=128))
            nc.sync.dma_start(xh_sb[:],
                              xh_e.rearrange("(n p) c -> p n c", p=128))
            nc.sync.dma_start(sc_sb[:], sc_e)
            nc.sync.dma_start(b1r_sb[:],
                              b1r_e.rearrange("(n p) -> p n", p=128))
            nc.sync.dma_start(b1s_sb[:],
                              b1s_e.rearrange("(n p) -> p n", p=128))

            def emit_qk(p, tbb):
                qp = psA.tile([128, 1536], f32, tag="psA", name="qp")[:, :1024]
                kp = psA.tile([128, 1536], f32, tag="psA", name="kp")[:, :1024]
                psl = slice(p * 128, (p + 1) * 128)
                for dst, w_sb, b_sb in ((qp, wq_sb, "bq"), (kp, wk_sb, "bk")):
                    for half in range(2):
                        t0 = tbb * 1024 + half * 512
                        sl = slice(half * 512, (half + 1) * 512)
                        for cc_ in range(4):
                            nc.tensor.matmul(
                                dst[:, sl],
                                lhsT=w_sb[:, cc_, psl],
                                rhs=hT[:, cc_, t0:t0 + 512],
                                start=(cc_ == 0),
                                stop=(cc_ == 3 and not has_bqkv),
                            )
                        if has_bqkv:
                            bsb = bq_sb if b_sb == "bq" else bk_sb
                            nc.tensor.matmul(
                                dst[:, sl],
                                lhsT=bsb[0:1, psl],
                                rhs=ones_sb[0:1, :],
                                start=False, stop=True, skip_group_check=True,
                            )
                tsl = slice(tbb * 1024, (tbb + 1) * 1024)
                nc.scalar.copy(qT[:, p, tsl], qp[:])
                nc.vector.tensor_copy(kT[:, p, tsl], kp[:])

            hms = []
            for m in range(NT):
                hm = hpool.tile([128, C], bf16, tag=f"hm{m}", name=f"hm{m}")
                layer_norm_tile(x_sb[:, m, :], hm[:])
                hms.append(hm)
            for m in range(NT):
                hm = hms[m]
                tp = psB.tile([128, 4, 128], bf16, tag="psB", name="tp")
                for cc_ in range(4):
                    nc.tensor.transpose(
                        tp[:, cc_, :],
                        hm[:, cc_ * 128:(cc_ + 1) * 128],
                        ident_b[:],
                    )
                msl = slice(m * 128, (m + 1) * 128)
                nc.scalar.copy(hT[:, :, msl], tp[:])
            for m in range(NT):
                msl = slice(m * 128, (m + 1) * 128)
                # v for this token tile (all 4 heads along free axis)
                vp_ = psB.tile([128, 512], f32, tag="psB",
                               name="vp")[:, :HPC * D]
                for cc_ in range(4):
                    nc.tensor.matmul(
                        vp_,
                        lhsT=hT[:, cc_, msl],
                        rhs=wv_sb[:, cc_, :],
                        start=(cc_ == 0),
                        stop=(cc_ == 3 and not has_bqkv),
                    )
                if has_bqkv:
                    nc.tensor.matmul(
                        vp_,
                        lhsT=ones_sb[0:1, :128],
                        rhs=bv_sb[0:1, :],
                        start=False, stop=True, skip_group_check=True,
                    )
                nc.vector.tensor_copy(v_sb[:, m, :], vp_)
                if m == 7:
                    emit_qk(0, 0)
                if m == 15:
                    emit_qk(0, 1)
            emit_qk(1, 0)
            emit_qk(1, 1)

        # ================= Phase B: attention =================
        h2mp = ctx.enter_context(tc.tile_pool(name="h2mp", bufs=1))
        h2ms = {}
        pts = {}

        def cf_front(k):
            """residual + LN2 for chunk k (pt data already landed)."""
            for mm_ in range(2):
                m = 2 * k + mm_
                pt = pts[m]
                nc.vector.tensor_tensor(x2[:, m, :], xh_sb[:, m, :], pt[:],
                                        ALU.add)
                if has_bo:
                    nc.vector.tensor_tensor(
                        x2[:, m, :], x2[:, m, :], bo_sb[:], ALU.add
                    )
                hm = h2mp.tile([128, C], bf16, tag=f"h2m{m}", name=f"h2m{m}")
                layer_norm_tile(x2[:, m, :], hm[:], dve=True)
                h2ms[m] = hm

        def cf_tp_m(m):
            """transposes + fp8 hi/lo h2T for one token tile."""
            if True:
                hm = h2ms[m]
                tp = psB.tile([128, 4, 128], bf16, tag="psB", name="tp2")
                for cc_ in range(4):
                    nc.tensor.transpose(
                        tp[:, cc_, :],
                        hm[:, cc_ * 128:(cc_ + 1) * 128],
                        ident_b[:],
                    )
                msl = slice(m * 128, (m + 1) * 128)
                nc.scalar.copy(h2T[0][:, :, msl], tp[:])
                nc.vector.tensor_tensor(
                    h2T[1][:, :, msl], tp[:], h2T[0][:, :, msl], ALU.subtract
                )

        def cf_tp(k):
            cf_tp_m(2 * k)
            cf_tp_m(2 * k + 1)

        def emit_wo(k):
            for mm_ in range(NT // NCH):
                m = k * (NT // NCH) + mm_
                msl = slice(m * 128, (m + 1) * 128)
                if k == 3 and mm_ % 2 == 1:
                    pp = psA.tile([128, 1536], f32, tag="psA",
                                  name="pp")[:, :512]
                else:
                    pp = psB.tile([128, 512], f32, tag="psB", name="pp")
                nmm = 0
                for aa, wa in ((0, 0), (0, 1), (1, 0)):
                    nmm += 1
                    asrc = attn_hi if aa == 0 else attn_lo
                    nc.tensor.matmul(
                        pp[:],
                        lhsT=asrc[:, :, msl],
                        rhs=wo_sb[wa][:],
                        start=(nmm == 1), stop=(nmm == 3),
                        perf_mode=DR,
                    )
                pj = smalls.tile([128, 512], bf16, tag="pj")
                nc.vector.tensor_scalar(pj[:], pp[:], wos, None,
                                        ALU.mult)
                nc.sync.dma_start(
                    cc_in[k].ap()[mm_ * 128:(mm_ + 1) * 128, :], pj[:]
                )
            if sim:
                nc.sync.dma_start(cc_out[k].ap(),
                                  cc_in[k].ap()[:T // NCH // 2, :])
            else:
                nc.gpsimd.collective_compute(
                    "ReduceScatter",
                    ALU.add,
                    replica_groups=[[0, 1], [2, 3], [4, 5], [6, 7]],
                    ins=[cc_in[k].ap()],
                    outs=[cc_out[k].ap()],
                )
            for mm_ in range(2):
                m = 2 * k + mm_
                pt = h2mp.tile([128, 512], bf16, tag=f"pr{m}",
                               name=f"pr{m}")
                nc.sync.dma_start(
                    pt[:], cc_out[k].ap()[mm_ * 128:(mm_ + 1) * 128, :]
                )
                pts[m] = pt

        expp = ctx.enter_context(tc.tile_pool(name="expp", bufs=1))
        vsp = ctx.enter_context(tc.tile_pool(name="vsp", bufs=1))
        zp = ctx.enter_context(tc.tile_pool(name="zp", bufs=2))

        if True:
            pending = []  # delayed AV emission closures

            for h in range(HPC):
                p, u = h // 2, h % 2
                usl = slice(64 * u, 64 * u + 64)
                z = zp.tile([128, NT], f32, tag=f"z{h % 2}")
                zr = zp.tile([128, NT], f32, tag=f"zr{h % 2}")
                vs = vsp.tile([128, NT, D], bf16, tag=f"vs{h % 2}")
                exps = []

                def make_av(h, j, p, u, usl, exps, vs):
                    def av_emit():
                        av = psB.tile([128, 512], f32, tag="psB",
                                      name=f"av_{h}_{j}")
                        for ii in range(4 * j + 4):
                            off = 128 * ii - 512 * j
                            if off <= 0:
                                nc.tensor.matmul(
                                    av[usl, :],
                                    lhsT=vs[:, ii, :],
                                    rhs=exps[ii][:, -off:-off + 512],
                                    start=(ii == 0), stop=(ii == 4 * j + 3),
                                )
                            else:
                                nc.tensor.matmul(
                                    av[usl, off:],
                                    lhsT=vs[:, ii, :],
                                    rhs=exps[ii][:, 0:512 - off],
                                    start=False, stop=(ii == 4 * j + 3),
                                    skip_group_check=True,
                                )
                        jsl = slice(j * 512, (j + 1) * 512)
                        nc.vector.tensor_scalar(
                            attn_hi[usl, p, jsl], av[usl, :], SA, None,
                            ALU.mult,
                        )
                        nc.vector.scalar_tensor_tensor(
                            attn_lo[usl, p, jsl], av[usl, :], SA,
                            attn_hi[usl, p, jsl], ALU.mult, ALU.subtract,
                        )
                        if h == 3:
                            emit_wo(j)
                            if j >= 2:
                                cf_front(j - 2)
                    return av_emit

                for i in range(NT):
                    t0 = 128 * i
                    blk = 512 * (i // 4)
                    et = expp.tile([128, T - t0], bf16,
                                   tag=f"exp{h % 2 if i < 2 else 2}_{i}",
                                   name=f"exp_{h}_{i}")
                    exps.append(et)
                    ps = psA.tile([128, 1536], f32, tag="psA",
                                  name=f"sc_{h}_{i}")
                    nblocks = min(3, 4 - i // 4)
                    for sb in range(nblocks):
                        tstart = blk + 512 * sb
                        seg_lo = max(t0, tstart)
                        nc.tensor.matmul(
                            ps[:, seg_lo - blk:tstart + 512 - blk],
                            lhsT=kT[usl, p, i * 128:(i + 1) * 128],
                            rhs=qT[usl, p, seg_lo:tstart + 512],
                            start=True, stop=(sb > 0),
                        )
                        if sb == 0:
                            off = t0 - blk
                            nc.tensor.matmul(
                                ps[:, off:off + 128],
                                lhsT=ident_b[:],
                                rhs=mb_b[:],
                                start=False, stop=True,
                                skip_group_check=True,
                            )
                    hi1 = min(blk + 1536, T)
                    if hi1 >= T and i >= 8 and h < 3:
                        nc.scalar.activation(
                            et[:, 0:T - t0], ps[:, t0 - blk:T - blk],
                            AF.Exp, bias=0.0, scale=qks,
                        )
                        nc.vector.tensor_reduce(
                            z[:, i:i + 1], et[:, 0:T - t0],
                            mybir.AxisListType.X, ALU.add,
                        )
                    elif hi1 >= T:
                        nc.scalar.activation(
                            et[:, 0:T - t0], ps[:, t0 - blk:T - blk],
                            AF.Exp, bias=0.0, scale=qks,
                            accum_out=z[:, i:i + 1],
                        )
                    else:
                        ps2 = psB.tile([128, 512], f32, tag="psB",
                                       name=f"sc2_{h}_{i}")
                        nc.tensor.matmul(
                            ps2[:, 0:512],
                            lhsT=kT[usl, p, i * 128:(i + 1) * 128],
                            rhs=qT[usl, p, 1536:2048],
                            start=True, stop=True,
                        )
                        zpt = zp.tile([128, 1], f32, tag="zpart")
                        nc.scalar.activation(
                            et[:, 0:hi1 - t0], ps[:, t0 - blk:hi1 - blk],
                            AF.Exp, bias=0.0, scale=qks, accum_out=zpt[:],
                        )
                        nc.scalar.activation(
                            et[:, 1536 - t0:2048 - t0], ps2[:, 0:512],
                            AF.Exp, bias=0.0, scale=qks,
                        )
                        zpt2 = zp.tile([128, 1], f32, tag="zpart2")
                        nc.vector.tensor_reduce(
                            zpt2[:], et[:, 1536 - t0:2048 - t0],
                            mybir.AxisListType.X, ALU.add,
                        )
                        nc.vector.tensor_tensor(
                            z[:, i:i + 1], zpt[:], zpt2[:], ALU.add
                        )
                    nc.vector.reciprocal(zr[:, i:i + 1], z[:, i:i + 1])
                    nc.vector.tensor_scalar(
                        vs[:, i, :], v_sb[:, i, h * D:(h + 1) * D],
                        zr[:, i:i + 1], vsc, ALU.mult, ALU.mult,
                    )
                    if i % 4 == 1 and pending:
                        pending.pop(0)()
                    if i % 4 == 3:
                        pending.append(make_av(h, i // 4, p, u, usl, exps, vs))
            _CACHE["_pending_drain"] = pending

        # ================= Phase C: FFN =================
        relup = ctx.enter_context(tc.tile_pool(name="relup", bufs=1))
        if True:
            rdt = f8 if FP8W2 else bf16
            rlt = [relup.tile([128, 16, 512], rdt, tag=f"rl_{tb}",
                              name=f"rl_{tb}") for tb in range(2)]

            def emit_w2_tile(tb, mloc):
                m = tb * 4 + mloc
                if tb == 1:
                    f2 = psA.tile([128, 1536], f32, tag="psA",
                                  name="f2")[:, :512]
                else:
                    f2 = psB.tile([128, 512], f32, tag="psB", name="f2")
                if FP8W2:
                    nmm = 0
                    for wa in range(2):
                        for j in range(8):
                            nmm += 1
                            nc.tensor.matmul(
                                f2[:],
                                lhsT=rlt[tb][:, 2 * j:2 * j + 2,
                                             mloc * 128:(mloc + 1) * 128],
                                rhs=w2_sb[wa][:, 2 * j:2 * j + 2, :],
                                start=(nmm == 1), stop=(nmm == 16),
                                perf_mode=DR,
                            )
                else:
                    for j in range(16):
                        nc.tensor.matmul(
                            f2[:],
                            lhsT=rlt[tb][:, j, mloc * 128:(mloc + 1) * 128],
                            rhs=w2_sb[:, j, :],
                            start=(j == 0), stop=(j == 15),
                        )
                yt = smalls.tile([128, 512], f32, tag="yt")
                if FP8W2:
                    nc.vector.scalar_tensor_tensor(
                        yt[:], f2[:], w2s, x2[:, m, :], ALU.mult, ALU.add
                    )
                else:
                    nc.vector.tensor_tensor(yt[:], f2[:], x2[:, m, :],
                                            ALU.add)
                if has_b2:
                    nc.vector.tensor_tensor(
                        yt[:], yt[:], b2_sb[:], ALU.add
                    )
                nc.sync.dma_start(y_e[m * 128:(m + 1) * 128, :], yt[:])

            def emit_w1(tb, interleave=None):
                for nn in range(16):
                    if interleave is not None and nn >= 8 and nn % 2 == 0:
                        emit_w2_tile(interleave, (nn - 8) // 2)
                    fp = psA.tile([128, 1536], f32, tag="psA",
                                  name="fp")[:, :512]
                    nmm = 0
                    for wa, ha in ((0, 0), (0, 1), (1, 0)):
                        for j in range(2):
                            nmm += 1
                            nc.tensor.matmul(
                                fp[:],
                                lhsT=w1_sb[wa][:, 2 * j:2 * j + 2,
                                               nn * 128:(nn + 1) * 128],
                                rhs=h2T[ha][:, 2 * j:2 * j + 2,
                                            tb * 512:(tb + 1) * 512],
                                start=(nmm == 1), stop=(nmm == 6),
                                perf_mode=DR,
                            )
                    if FP8W2:
                        # rl = SR*relu_true in fp8
                        nc.scalar.activation(
                            rlt[tb][:, nn, :], fp[:], AF.Relu,
                            bias=b1s_sb[:, nn:nn + 1], scale=rsc,
                        )
                    elif nn % 2 == 0:
                        # rl = SH*c1*relu_true; W2 is pre-divided by SH*c1
                        nc.vector.tensor_scalar(
                            rlt[tb][:, nn, :], fp[:],
                            b1r_sb[:, nn:nn + 1], 0.0, ALU.add, ALU.max,
                        )
                    else:
                        nc.scalar.activation(
                            rlt[tb][:, nn, :], fp[:], AF.Relu,
                            bias=b1r_sb[:, nn:nn + 1], scale=1.0,
                        )

            def emit_w2(tb):
                for mloc in range(4):
                    emit_w2_tile(tb, mloc)

            drain = _CACHE.pop("_pending_drain")
            while drain:
                drain.pop(0)()      # AV(h3,3) + Wo(3) + cf_front(1)
            cf_tp(0)
            cf_tp(1)
            cf_front(2)
            cf_front(3)
            emit_w1(0)
            cf_tp(2)
            cf_tp(3)
            emit_w1(1, interleave=0)
            emit_w2(1)

    nc.compile()
    return nc


# revision 60
# speedup vs baseline: 1.0144x; 1.0144x over previous
"""Trainium2 Bass kernel for a single pre-norm transformer block.

Reference math (B=4, T=2048, C=512, H=8, D=64, fp32):
    h  = LN(x; g1, b1) ; q,k,v = h @ Wq/Wk/Wv (per head)
    wei = softmax_over_QUERY_axis( causal_mask(q k^T / sqrt(C)) )
    x2  = x + concat_heads(wei @ v) @ Wo + bo
    out = x2 + relu(LN(x2; g2, b2) @ W1 + b1) @ W2 + b2

Sharding over 8 NeuronCores: pairs of cores per batch element
(core = 2*b + r). Each core computes LN1 + QKV for its batch,
attention for its 4 heads (h = 4r..4r+3), and the partial output
projection (contracting only its heads' features). A pairwise
ReduceScatter sums the partial projections and hands each core its
half of the tokens; the FFN is token-parallel (1024 tokens/core).

Numerics: W1 and Wo run on fp8e4m3 operands in DoubleRow perf mode
(K=256 per instruction at 0.5 cyc/row), with hi+lo fp8 splits of both
operands (x ~ hi + lo keeps ~7 mantissa bits; 3 cross products) so
the error stays at bf16 level. W2 contracts fp8 relu activations
(single quantization, the dominant error term at ~1.4e-2 rel) against
a hi+lo split of W2. QKV, attention scores and AV stay bf16. The
per-tensor power-of-2 scales are absorbed by the exp's scale operand
and the epilogue multiplies, so no extra passes are spent on
rescaling. Engine budget: ACT is reserved for the softmax exp during
attention (the hard floor at ~73us/core); LN/copy/relu traffic is
spread across DVE/Pool/ACT by phase; Wo + ReduceScatter + LN2 + the
first FFN half are pipelined into head 3's attention tail.
"""

import sys

sys.path.insert(0, "/opt/trn_rl_repo")

import ml_dtypes
import numpy as np

B, T, C, H, D = 4, 2048, 512, 8, 64
EPS = 1e-5
NCORES = 8
TH = T // 2  # tokens per core in the FFN phase
HPC = H // 2  # heads per core
NT = T // 128  # 16 token tiles per batch
NEG = -1e30
NCH = 4  # collective chunks

SH = 32.0  # scale on LN outputs before fp8 (|h| <= ~5.2 -> 166 < 240)
SA = 16.0  # scale on attention outputs before fp8
SR = 16.0  # scale on relu outputs before fp8

E4NP = ml_dtypes.float8_e4m3
FP8W2 = True  # fp8 relu activations against split-fp8 W2 (vs all-bf16 W2)

_CACHE: dict = {}


def _build_program(flags, sim=False):
    from contextlib import ExitStack

    import concourse.bacc as bacc
    import concourse.bass as bass
    import concourse.tile as tile
    from concourse import mybir
    from concourse.masks import make_identity

    has_bqkv, has_bo, has_b2 = flags
    f32 = mybir.dt.float32
    bf16 = mybir.dt.bfloat16
    f8 = mybir.dt.float8e4
    AF = mybir.ActivationFunctionType
    ALU = mybir.AluOpType
    DR = mybir.MatmulPerfMode.DoubleRow

    nc = bacc.Bacc(
        "TRN2", target_bir_lowering=False, debug=False,
        num_devices=1 if sim else NCORES,
    )

    x_e = nc.dram_tensor("x", [T, C], bf16, kind="ExternalInput").ap()
    xh_e = nc.dram_tensor("xh", [TH, C], bf16, kind="ExternalInput").ap()
    wq_e = nc.dram_tensor("wq", [C, HPC * D], bf16,
                          kind="ExternalInput").ap()
    wk_e = nc.dram_tensor("wk", [C, HPC * D], bf16,
                          kind="ExternalInput").ap()
    wv_e = nc.dram_tensor("wv", [C, HPC * D], bf16,
                          kind="ExternalInput").ap()
    wo_es = [nc.dram_tensor(f"wo{s}", [128, 2, C], f8,
                            kind="ExternalInput").ap() for s in "hl"]
    w1_es = [nc.dram_tensor(f"w1{s}", [C, 4 * C], f8,
                            kind="ExternalInput").ap() for s in "hl"]
    if FP8W2:
        w2_es = [nc.dram_tensor(f"w2{s}", [4 * C, C], f8,
                                kind="ExternalInput").ap() for s in "hl"]
    else:
        w2_e = nc.dram_tensor("w2", [4 * C, C], bf16,
                              kind="ExternalInput").ap()
    b1r_e = nc.dram_tensor("b1r", [4 * C], f32, kind="ExternalInput").ap()
    b1s_e = nc.dram_tensor("b1s", [4 * C], f32, kind="ExternalInput").ap()
    # scales: col 0 qks=1/(SH^2 cq ck), 1 vsc=1/(SH cv), 2 wos=1/(SA cwo),
    #         3 rsc=SR/(SH c1), 4 w2s=1/(SR c2)
    sc_e = nc.dram_tensor("scales", [128, 8], f32, kind="ExternalInput").ap()
    if has_bqkv:
        bq_e = nc.dram_tensor("bq", [1, HPC * D], f32, kind="ExternalInput").ap()
        bk_e = nc.dram_tensor("bk", [1, HPC * D], f32, kind="ExternalInput").ap()
        bv_e = nc.dram_tensor("bv", [1, HPC * D], f32, kind="ExternalInput").ap()
    if has_bo:
        bo_e = nc.dram_tensor("bo", [C], f32, kind="ExternalInput").ap()
    if has_b2:
        b2_e = nc.dram_tensor("b2", [C], f32, kind="ExternalInput").ap()
    y_e = nc.dram_tensor("y", [TH, C], f32, kind="ExternalOutput").ap()

    cc_in = [nc.dram_tensor(f"cc_in{k}", [T // NCH, C], bf16)
             for k in range(NCH)]
    cc_out = [nc.dram_tensor(f"cc_out{k}", [T // NCH // 2, C], bf16)
              for k in range(NCH)]

    with tile.TileContext(nc) as tc, ExitStack() as ctx:
        psA = ctx.enter_context(tc.tile_pool(name="psA", bufs=2, space="PSUM"))
        psB = ctx.enter_context(tc.tile_pool(name="psB", bufs=2, space="PSUM"))
        consts = ctx.enter_context(tc.tile_pool(name="consts", bufs=1))
        smalls = ctx.enter_context(tc.tile_pool(name="smalls", bufs=2))
        qk_pool = ctx.enter_context(tc.tile_pool(name="qk", bufs=1))
        vpool = ctx.enter_context(tc.tile_pool(name="vp", bufs=1))
        attnp = ctx.enter_context(tc.tile_pool(name="attnp", bufs=1))
        fw = ctx.enter_context(tc.tile_pool(name="fw", bufs=1))
        x2p = ctx.enter_context(tc.tile_pool(name="x2p", bufs=1))
        h2p = ctx.enter_context(tc.tile_pool(name="h2p", bufs=1))

        # ---- constants ----
        ident_b = consts.tile([128, 128], bf16)
        make_identity(nc, ident_b)
        mb_b = consts.tile([128, 128], bf16)
        nc.gpsimd.memset(mb_b[:], 0.0)
        nc.gpsimd.affine_select(
            out=mb_b[:], in_=mb_b[:], compare_op=ALU.is_ge, fill=NEG,
            base=0, pattern=[[1, 128]], channel_multiplier=-1,
        )
        eps_t = consts.tile([128, 1], f32)
        nc.vector.memset(eps_t[:], EPS / (SH * SH))
        b1r_sb = consts.tile([128, 16], f32)
        b1s_sb = consts.tile([128, 16], f32)
        sc_sb = consts.tile([128, 8], f32)
        qks = sc_sb[:, 0:1]
        vsc = sc_sb[:, 1:2]
        wos = sc_sb[:, 2:3]
        rsc = sc_sb[:, 3:4]
        w2s = sc_sb[:, 4:5]
        if has_bqkv:
            ones_sb = consts.tile([1, 512], f32)
            nc.vector.memset(ones_sb[:], 1.0)
            bq_sb = consts.tile([1, HPC * D], f32)
            nc.sync.dma_start(bq_sb[:], bq_e)
            bk_sb = consts.tile([1, HPC * D], f32)
            nc.sync.dma_start(bk_sb[:], bk_e)
            bv_sb = consts.tile([1, HPC * D], f32)
            nc.sync.dma_start(bv_sb[:], bv_e)
        if has_bo:
            bo_sb = consts.tile([128, C], f32)
            bo_b = bo_e[None, :]
            bo_bc = bass.AP(
                tensor=bo_b.tensor, offset=bo_b.offset,
                ap=[[0, 128], bo_b.ap[1]],
            )
            nc.sync.dma_start(bo_sb[:], bo_bc)
        if has_b2:
            b2_sb = consts.tile([128, C], f32)
            b2_b = b2_e[None, :]
            b2_bc = bass.AP(
                tensor=b2_b.tensor, offset=b2_b.offset,
                ap=[[0, 128], b2_b.ap[1]],
            )
            nc.sync.dma_start(b2_sb[:], b2_bc)

        # persistent activations
        qT = qk_pool.tile([128, 2, T], bf16)  # [pair-head d, pair, t]
        kT = qk_pool.tile([128, 2, T], bf16)
        wq_sb = qk_pool.tile([128, 4, HPC * D], bf16)
        wk_sb = qk_pool.tile([128, 4, HPC * D], bf16)
        hT = qk_pool.tile([128, 4, T], bf16)
        v_sb = vpool.tile([128, NT, HPC * D], bf16)  # [s in tile, tile, hd]
        attn_hi = attnp.tile([128, 2, T], f8)  # [hd in pair, pair, t] * SA
        attn_lo = attnp.tile([128, 2, T], f8)

        # FFN weights + residual stream (DMA emitted inside phase A, after
        # the x loads, so the x tiles win the DMA queue)
        w1_sb = [fw.tile([128, 4, 4 * C], f8, name=f"w1_{s}")
                 for s in range(2)]
        if FP8W2:
            w2_sb = [fw.tile([128, 16, C], f8, name=f"w2_{s}")
                     for s in range(2)]
        else:
            w2_sb = fw.tile([128, 16, C], bf16)
        wo_sb = [fw.tile([128, 2, C], f8, name=f"wo_{s}")
                 for s in range(2)]
        xh_sb = fw.tile([128, 8, C], bf16)
        x2 = x2p.tile([128, 8, C], f32)
        h2T = [h2p.tile([128, 4, TH], f8, name=f"h2T_{s}")
               for s in range(2)]

        def layer_norm_tile(xm, hm, dve=False):
            """hm = SH * (xm - mean) * rsqrt(var + eps)."""
            stats = smalls.tile([128, 6], f32, tag="bnst")
            nc.vector.bn_stats(stats[:], xm)
            mv = smalls.tile([128, 2], f32, tag="bnag")
            nc.vector.bn_aggr(mv[:], stats[:])
            rstd = smalls.tile([128, 1], f32, tag="rstd")
            nc.scalar.activation(rstd[:], mv[:, 1:2], AF.Sqrt,
                                 bias=eps_t[:], scale=1.0 / (SH * SH))
            nc.vector.reciprocal(rstd[:], rstd[:])
            if dve:
                nc.vector.tensor_scalar(
                    hm, xm, mv[:, 0:1], rstd[:], ALU.subtract, ALU.mult
                )
                return
            nmr = smalls.tile([128, 1], f32, tag="nmr")
            nc.vector.tensor_scalar(
                nmr[:], mv[:, 0:1], rstd[:], -1.0, ALU.mult, ALU.mult
            )
            nc.scalar.activation(hm, xm, AF.Identity, bias=nmr[:],
                                 scale=rstd[:])

        # ================= Phase A: LN1 + QKV =================
        with ExitStack() as phaseA:
            wv_pool = phaseA.enter_context(tc.tile_pool(name="wvp", bufs=1))
            wv_sb = wv_pool.tile([128, 4, HPC * D], bf16)

            xpool = phaseA.enter_context(tc.tile_pool(name="xp", bufs=1))
            hpool = phaseA.enter_context(tc.tile_pool(name="hn", bufs=1))
            x_sb = xpool.tile([128, NT, C], bf16)
            x_r = x_e.rearrange("(n p) c -> p n c", p=128)
            for xc in range(16):
                nc.sync.dma_start(
                    x_sb[:, xc:xc + 1, :], x_r[:, xc:xc + 1, :],
                )
            nc.sync.dma_start(
                wq_sb[:], wq_e.rearrange("(o p) d -> p o d", p=128))
            nc.sync.dma_start(
                wk_sb[:], wk_e.rearrange("(o p) d -> p o d", p=128))
            nc.sync.dma_start(
                wv_sb[:], wv_e.rearrange("(o p) d -> p o d", p=128))
            for s in range(2):
                nc.sync.dma_start(
                    w1_sb[s][:], w1_es[s].rearrange("(o p) n -> p o n", p=128))
            for s in range(2):
                nc.sync.dma_start(wo_sb[s][:], wo_es[s])
            if FP8W2:
                for s in range(2):
                    nc.sync.dma_start(
                        w2_sb[s][:],
                        w2_es[s].rearrange("(o p) c -> p o c", p=128))
            else:
                nc.sync.dma_start(
                    w2_sb[:], w2_e.rearrange("(o p) c -> p o c", p=128))
            nc.sync.dma_start(xh_sb[:],
                              xh_e.rearrange("(n p) c -> p n c", p=128))
            nc.sync.dma_start(sc_sb[:], sc_e)
            nc.sync.dma_start(b1r_sb[:],
                              b1r_e.rearrange("(n p) -> p n", p=128))
            nc.sync.dma_start(b1s_sb[:],
                              b1s_e.rearrange("(n p) -> p n", p=128))

            def emit_qk(p, tbb, copy_dve=False, only=None):
                psl = slice(p * 128, (p + 1) * 128)
                pairs = []
                if only in (None, "q"):
                    qp = psA.tile([128, 1536], f32, tag="psA",
                                  name="qp")[:, :1024]
                    pairs.append((qp, wq_sb, "bq"))
                if only in (None, "k"):
                    kp = psA.tile([128, 1536], f32, tag="psA",
                                  name="kp")[:, :1024]
                    pairs.append((kp, wk_sb, "bk"))
                for dst, w_sb, b_sb in pairs:
                    for half in range(2):
                        t0 = tbb * 1024 + half * 512
                        sl = slice(half * 512, (half + 1) * 512)
                        for cc_ in range(4):
                            nc.tensor.matmul(
                                dst[:, sl],
                                lhsT=w_sb[:, cc_, psl],
                                rhs=hT[:, cc_, t0:t0 + 512],
                                start=(cc_ == 0),
                                stop=(cc_ == 3 and not has_bqkv),
                            )
                        if has_bqkv:
                            bsb = bq_sb if b_sb == "bq" else bk_sb
                            nc.tensor.matmul(
                                dst[:, sl],
                                lhsT=bsb[0:1, psl],
                                rhs=ones_sb[0:1, :],
                                start=False, stop=True, skip_group_check=True,
                            )
                tsl = slice(tbb * 1024, (tbb + 1) * 1024)
                if only in (None, "q"):
                    if copy_dve:
                        nc.vector.tensor_copy(qT[:, p, tsl], qp[:])
                    else:
                        nc.scalar.copy(qT[:, p, tsl], qp[:])
                if only in (None, "k"):
                    nc.vector.tensor_copy(kT[:, p, tsl], kp[:])

            hms = []
            for m in range(NT):
                hm = hpool.tile([128, C], bf16, tag=f"hm{m}", name=f"hm{m}")
                layer_norm_tile(x_sb[:, m, :], hm[:])
                hms.append(hm)
            for m in range(NT):
                hm = hms[m]
                tp = psB.tile([128, 4, 128], bf16, tag="psB", name="tp")
                for cc_ in range(4):
                    nc.tensor.transpose(
                        tp[:, cc_, :],
                        hm[:, cc_ * 128:(cc_ + 1) * 128],
                        ident_b[:],
                    )
                msl = slice(m * 128, (m + 1) * 128)
                nc.scalar.copy(hT[:, :, msl], tp[:])
            for m in range(NT):
                msl = slice(m * 128, (m + 1) * 128)
                # v for this token tile (all 4 heads along free axis)
                vp_ = psB.tile([128, 512], f32, tag="psB",
                               name="vp")[:, :HPC * D]
                for cc_ in range(4):
                    nc.tensor.matmul(
                        vp_,
                        lhsT=hT[:, cc_, msl],
                        rhs=wv_sb[:, cc_, :],
                        start=(cc_ == 0),
                        stop=(cc_ == 3 and not has_bqkv),
                    )
                if has_bqkv:
                    nc.tensor.matmul(
                        vp_,
                        lhsT=ones_sb[0:1, :128],
                        rhs=bv_sb[0:1, :],
                        start=False, stop=True, skip_group_check=True,
                    )
                nc.vector.tensor_copy(v_sb[:, m, :], vp_)
                if m == 7:
                    emit_qk(0, 0)
                if m == 15:
                    emit_qk(0, 1)
            _CACHE["_qk_defer"] = [
                lambda: emit_qk(1, 0, copy_dve=True, only="q"),
                lambda: emit_qk(1, 0, copy_dve=True, only="k"),
                lambda: emit_qk(1, 1, copy_dve=True, only="q"),
                lambda: emit_qk(1, 1, copy_dve=True, only="k"),
            ]

        # ================= Phase B: attention =================
        h2mp = ctx.enter_context(tc.tile_pool(name="h2mp", bufs=1))
        h2ms = {}
        pts = {}

        def cf_front(k):
            """residual + LN2 for chunk k (pt data already landed)."""
            for mm_ in range(2):
                m = 2 * k + mm_
                pt = pts[m]
                nc.vector.tensor_tensor(x2[:, m, :], xh_sb[:, m, :], pt[:],
                                        ALU.add)
                if has_bo:
                    nc.vector.tensor_tensor(
                        x2[:, m, :], x2[:, m, :], bo_sb[:], ALU.add
                    )
                hm = h2mp.tile([128, C], bf16, tag=f"h2m{m}", name=f"h2m{m}")
                layer_norm_tile(x2[:, m, :], hm[:], dve=True)
                h2ms[m] = hm

        def cf_tp_m(m):
            """transposes + fp8 hi/lo h2T for one token tile."""
            if True:
                hm = h2ms[m]
                tp = psB.tile([128, 4, 128], bf16, tag="psB", name="tp2")
                for cc_ in range(4):
                    nc.tensor.transpose(
                        tp[:, cc_, :],
                        hm[:, cc_ * 128:(cc_ + 1) * 128],
                        ident_b[:],
                    )
                msl = slice(m * 128, (m + 1) * 128)
                nc.scalar.copy(h2T[0][:, :, msl], tp[:])
                nc.vector.tensor_tensor(
                    h2T[1][:, :, msl], tp[:], h2T[0][:, :, msl], ALU.subtract
                )

        def cf_tp(k):
            cf_tp_m(2 * k)
            cf_tp_m(2 * k + 1)

        def emit_wo(k):
            for mm_ in range(NT // NCH):
                m = k * (NT // NCH) + mm_
                msl = slice(m * 128, (m + 1) * 128)
                if k == 3 and mm_ % 2 == 1:
                    pp = psA.tile([128, 1536], f32, tag="psA",
                                  name="pp")[:, :512]
                else:
                    pp = psB.tile([128, 512], f32, tag="psB", name="pp")
                nmm = 0
                for aa, wa in ((0, 0), (0, 1), (1, 0)):
                    nmm += 1
                    asrc = attn_hi if aa == 0 else attn_lo
                    nc.tensor.matmul(
                        pp[:],
                        lhsT=asrc[:, :, msl],
                        rhs=wo_sb[wa][:],
                        start=(nmm == 1), stop=(nmm == 3),
                        perf_mode=DR,
                    )
                pj = smalls.tile([128, 512], bf16, tag="pj")
                nc.vector.tensor_scalar(pj[:], pp[:], wos, None,
                                        ALU.mult)
                nc.sync.dma_start(
                    cc_in[k].ap()[mm_ * 128:(mm_ + 1) * 128, :], pj[:]
                )
            if sim:
                nc.sync.dma_start(cc_out[k].ap(),
                                  cc_in[k].ap()[:T // NCH // 2, :])
            else:
                nc.gpsimd.collective_compute(
                    "ReduceScatter",
                    ALU.add,
                    replica_groups=[[0, 1], [2, 3], [4, 5], [6, 7]],
                    ins=[cc_in[k].ap()],
                    outs=[cc_out[k].ap()],
                )
            for mm_ in range(2):
                m = 2 * k + mm_
                pt = h2mp.tile([128, 512], bf16, tag=f"pr{m}",
                               name=f"pr{m}")
                nc.sync.dma_start(
                    pt[:], cc_out[k].ap()[mm_ * 128:(mm_ + 1) * 128, :]
                )
                pts[m] = pt

        expp = ctx.enter_context(tc.tile_pool(name="expp", bufs=1))
        vsp = ctx.enter_context(tc.tile_pool(name="vsp", bufs=1))
        zp = ctx.enter_context(tc.tile_pool(name="zp", bufs=2))

        if True:
            pending = []  # delayed AV emission closures

            for h in range(HPC):
                p, u = h // 2, h % 2
                usl = slice(64 * u, 64 * u + 64)
                z = zp.tile([128, NT], f32, tag=f"z{h % 2}")
                zr = zp.tile([128, NT], f32, tag=f"zr{h % 2}")
                vs = vsp.tile([128, NT, D], bf16, tag=f"vs{h % 2}")
                exps = []

                def make_av(h, j, p, u, usl, exps, vs):
                    def av_emit():
                        av = psB.tile([128, 512], f32, tag="psB",
                                      name=f"av_{h}_{j}")
                        for ii in range(4 * j + 4):
                            off = 128 * ii - 512 * j
                            if off <= 0:
                                nc.tensor.matmul(
                                    av[usl, :],
                                    lhsT=vs[:, ii, :],
                                    rhs=exps[ii][:, -off:-off + 512],
                                    start=(ii == 0), stop=(ii == 4 * j + 3),
                                )
                            else:
                                nc.tensor.matmul(
                                    av[usl, off:],
                                    lhsT=vs[:, ii, :],
                                    rhs=exps[ii][:, 0:512 - off],
                                    start=False, stop=(ii == 4 * j + 3),
                                    skip_group_check=True,
                                )
                        jsl = slice(j * 512, (j + 1) * 512)
                        nc.vector.tensor_scalar(
                            attn_hi[usl, p, jsl], av[usl, :], SA, None,
                            ALU.mult,
                        )
                        nc.vector.scalar_tensor_tensor(
                            attn_lo[usl, p, jsl], av[usl, :], SA,
                            attn_hi[usl, p, jsl], ALU.mult, ALU.subtract,
                        )
                        if h == 3:
                            emit_wo(j)
                            if j >= 2:
                                cf_front(j - 2)
                    return av_emit

                for i in range(NT):
                    t0 = 128 * i
                    blk = 512 * (i // 4)
                    et = expp.tile([128, T - t0], bf16,
                                   tag=f"exp{h % 2 if i < 2 else 2}_{i}",
                                   name=f"exp_{h}_{i}")
                    exps.append(et)
                    ps = psA.tile([128, 1536], f32, tag="psA",
                                  name=f"sc_{h}_{i}")
                    nblocks = min(3, 4 - i // 4)
                    for sb in range(nblocks):
                        tstart = blk + 512 * sb
                        seg_lo = max(t0, tstart)
                        nc.tensor.matmul(
                            ps[:, seg_lo - blk:tstart + 512 - blk],
                            lhsT=kT[usl, p, i * 128:(i + 1) * 128],
                            rhs=qT[usl, p, seg_lo:tstart + 512],
                            start=True, stop=(sb > 0),
                        )
                        if sb == 0:
                            off = t0 - blk
                            nc.tensor.matmul(
                                ps[:, off:off + 128],
                                lhsT=ident_b[:],
                                rhs=mb_b[:],
                                start=False, stop=True,
                                skip_group_check=True,
                            )
                    hi1 = min(blk + 1536, T)
                    if hi1 >= T and i >= 8 and h < 3:
                        nc.scalar.activation(
                            et[:, 0:T - t0], ps[:, t0 - blk:T - blk],
                            AF.Exp, bias=0.0, scale=qks,
                        )
                        nc.vector.tensor_reduce(
                            z[:, i:i + 1], et[:, 0:T - t0],
                            mybir.AxisListType.X, ALU.add,
                        )
                    elif hi1 >= T:
                        nc.scalar.activation(
                            et[:, 0:T - t0], ps[:, t0 - blk:T - blk],
                            AF.Exp, bias=0.0, scale=qks,
                            accum_out=z[:, i:i + 1],
                        )
                    else:
                        ps2 = psB.tile([128, 512], f32, tag="psB",
                                       name=f"sc2_{h}_{i}")
                        nc.tensor.matmul(
                            ps2[:, 0:512],
                            lhsT=kT[usl, p, i * 128:(i + 1) * 128],
                            rhs=qT[usl, p, 1536:2048],
                            start=True, stop=True,
                        )
                        zpt = zp.tile([128, 1], f32, tag="zpart")
                        nc.scalar.activation(
                            et[:, 0:hi1 - t0], ps[:, t0 - blk:hi1 - blk],
                            AF.Exp, bias=0.0, scale=qks, accum_out=zpt[:],
                        )
                        nc.scalar.activation(
                            et[:, 1536 - t0:2048 - t0], ps2[:, 0:512],
                            AF.Exp, bias=0.0, scale=qks,
                        )
                        zpt2 = zp.tile([128, 1], f32, tag="zpart2")
                        nc.vector.tensor_reduce(
                            zpt2[:], et[:, 1536 - t0:2048 - t0],
                            mybir.AxisListType.X, ALU.add,
                        )
                        nc.vector.tensor_tensor(
                            z[:, i:i + 1], zpt[:], zpt2[:], ALU.add
                        )
                    nc.vector.reciprocal(zr[:, i:i + 1], z[:, i:i + 1])
                    nc.vector.tensor_scalar(
                        vs[:, i, :], v_sb[:, i, h * D:(h + 1) * D],
                        zr[:, i:i + 1], vsc, ALU.mult, ALU.mult,
                    )
                    if h == 0 and i in (3, 7, 11) or (h, i) == (1, 3):
                        _CACHE["_qk_defer"].pop(0)()
                    if i % 4 == 1 and pending:
                        pending.pop(0)()
                    if i % 4 == 3:
                        pending.append(make_av(h, i // 4, p, u, usl, exps, vs))
            _CACHE["_pending_drain"] = pending

        # ================= Phase C: FFN =================
        relup = ctx.enter_context(tc.tile_pool(name="relup", bufs=1))
        if True:
            rdt = f8 if FP8W2 else bf16
            rlt = [relup.tile([128, 16, 512], rdt, tag=f"rl_{tb}",
                              name=f"rl_{tb}") for tb in range(2)]

            def emit_w2_tile(tb, mloc):
                m = tb * 4 + mloc
                if tb == 1:
                    f2 = psA.tile([128, 1536], f32, tag="psA",
                                  name="f2")[:, :512]
                else:
                    f2 = psB.tile([128, 512], f32, tag="psB", name="f2")
                if FP8W2:
                    nmm = 0
                    for wa in range(2):
                        for j in range(8):
                            nmm += 1
                            nc.tensor.matmul(
                                f2[:],
                                lhsT=rlt[tb][:, 2 * j:2 * j + 2,
                                             mloc * 128:(mloc + 1) * 128],
                                rhs=w2_sb[wa][:, 2 * j:2 * j + 2, :],
                                start=(nmm == 1), stop=(nmm == 16),
                                perf_mode=DR,
                            )
                else:
                    for j in range(16):
                        nc.tensor.matmul(
                            f2[:],
                            lhsT=rlt[tb][:, j, mloc * 128:(mloc + 1) * 128],
                            rhs=w2_sb[:, j, :],
                            start=(j == 0), stop=(j == 15),
                        )
                yt = smalls.tile([128, 512], f32, tag="yt")
                if FP8W2:
                    nc.vector.scalar_tensor_tensor(
                        yt[:], f2[:], w2s, x2[:, m, :], ALU.mult, ALU.add
                    )
                else:
                    nc.vector.tensor_tensor(yt[:], f2[:], x2[:, m, :],
                                            ALU.add)
                if has_b2:
                    nc.vector.tensor_tensor(
                        yt[:], yt[:], b2_sb[:], ALU.add
                    )
                nc.sync.dma_start(y_e[m * 128:(m + 1) * 128, :], yt[:])

            def emit_w1(tb, interleave=None):
                for nn in range(16):
                    if interleave is not None and nn >= 8 and nn % 2 == 0:
                        emit_w2_tile(interleave, (nn - 8) // 2)
                    fp = psA.tile([128, 1536], f32, tag="psA",
                                  name="fp")[:, :512]
                    nmm = 0
                    for wa, ha in ((0, 0), (0, 1), (1, 0)):
                        for j in range(2):
                            nmm += 1
                            nc.tensor.matmul(
                                fp[:],
                                lhsT=w1_sb[wa][:, 2 * j:2 * j + 2,
                                               nn * 128:(nn + 1) * 128],
                                rhs=h2T[ha][:, 2 * j:2 * j + 2,
                                            tb * 512:(tb + 1) * 512],
                                start=(nmm == 1), stop=(nmm == 6),
                                perf_mode=DR,
                            )
                    if FP8W2:
                        # rl = SR*relu_true in fp8
                        nc.scalar.activation(
                            rlt[tb][:, nn, :], fp[:], AF.Relu,
                            bias=b1s_sb[:, nn:nn + 1], scale=rsc,
                        )
                    elif nn % 2 == 0:
                        # rl = SH*c1*relu_true; W2 is pre-divided by SH*c1
                        nc.vector.tensor_scalar(
                            rlt[tb][:, nn, :], fp[:],
                            b1r_sb[:, nn:nn + 1], 0.0, ALU.add, ALU.max,
                        )
                    else:
                        nc.scalar.activation(
                            rlt[tb][:, nn, :], fp[:], AF.Relu,
                            bias=b1r_sb[:, nn:nn + 1], scale=1.0,
                        )

            def emit_w2(tb):
                for mloc in range(4):
                    emit_w2_tile(tb, mloc)

            drain = _CACHE.pop("_pending_drain")
            while drain:
                drain.pop(0)()      # AV(h3,3) + Wo(3) + cf_front(1)
            cf_tp(0)
            cf_tp(1)
            cf_front(2)
            cf_front(3)
            emit_w1(0)
            cf_tp(2)
            cf_tp(3)
            emit_w1(1, interleave=0)
            emit_w2(1)

    nc.compile()
    return nc


# revision 64
# speedup vs baseline: 1.0154x; 1.0010x over previous
"""Trainium2 Bass kernel for a single pre-norm transformer block.

Reference math (B=4, T=2048, C=512, H=8, D=64, fp32):
    h  = LN(x; g1, b1) ; q,k,v = h @ Wq/Wk/Wv (per head)
    wei = softmax_over_QUERY_axis( causal_mask(q k^T / sqrt(C)) )
    x2  = x + concat_heads(wei @ v) @ Wo + bo
    out = x2 + relu(LN(x2; g2, b2) @ W1 + b1) @ W2 + b2

Sharding over 8 NeuronCores: pairs of cores per batch element
(core = 2*b + r). Each core computes LN1 + QKV for its batch,
attention for its 4 heads (h = 4r..4r+3), and the partial output
projection (contracting only its heads' features). A pairwise
ReduceScatter sums the partial projections and hands each core its
half of the tokens; the FFN is token-parallel (1024 tokens/core).

Numerics: W1 and Wo run on fp8e4m3 operands in DoubleRow perf mode
(K=256 per instruction at 0.5 cyc/row), with hi+lo fp8 splits of both
operands (x ~ hi + lo keeps ~7 mantissa bits; 3 cross products) so
the error stays at bf16 level. W2 contracts fp8 relu activations
(single quantization, the dominant error term at ~1.4e-2 rel) against
a hi+lo split of W2. QKV, attention scores and AV stay bf16. The
per-tensor power-of-2 scales are absorbed by the exp's scale operand
and the epilogue multiplies, so no extra passes are spent on
rescaling. Engine budget: ACT is reserved for the softmax exp during
attention (the hard floor at ~73us/core); LN/copy/relu traffic is
spread across DVE/Pool/ACT by phase; Wo + ReduceScatter + LN2 + the
first FFN half are pipelined into head 3's attention tail.
"""

import sys

sys.path.insert(0, "/opt/trn_rl_repo")

import ml_dtypes
import numpy as np

B, T, C, H, D = 4, 2048, 512, 8, 64
EPS = 1e-5
NCORES = 8
TH = T // 2  # tokens per core in the FFN phase
HPC = H // 2  # heads per core
NT = T // 128  # 16 token tiles per batch
NEG = -1e30
NCH = 4  # collective chunks

SH = 32.0  # scale on LN outputs before fp8 (|h| <= ~5.2 -> 166 < 240)
SA = 16.0  # scale on attention outputs before fp8
SR = 16.0  # scale on relu outputs before fp8

E4NP = ml_dtypes.float8_e4m3
FP8W2 = True  # fp8 relu activations against split-fp8 W2 (vs all-bf16 W2)

_CACHE: dict = {}


def _build_program(flags, sim=False):
    from contextlib import ExitStack

    import concourse.bacc as bacc
    import concourse.bass as bass
    import concourse.tile as tile
    from concourse import mybir
    from concourse.masks import make_identity

    has_bqkv, has_bo, has_b2 = flags
    f32 = mybir.dt.float32
    bf16 = mybir.dt.bfloat16
    f8 = mybir.dt.float8e4
    AF = mybir.ActivationFunctionType
    ALU = mybir.AluOpType
    DR = mybir.MatmulPerfMode.DoubleRow

    nc = bacc.Bacc(
        "TRN2", target_bir_lowering=False, debug=False,
        num_devices=1 if sim else NCORES,
    )

    x_e = nc.dram_tensor("x", [T, C], bf16, kind="ExternalInput").ap()
    xh_e = nc.dram_tensor("xh", [TH, C], bf16, kind="ExternalInput").ap()
    wq_e = nc.dram_tensor("wq", [C, HPC * D], bf16,
                          kind="ExternalInput").ap()
    wk_e = nc.dram_tensor("wk", [C, HPC * D], bf16,
                          kind="ExternalInput").ap()
    wv_e = nc.dram_tensor("wv", [C, HPC * D], bf16,
                          kind="ExternalInput").ap()
    wo_es = [nc.dram_tensor(f"wo{s}", [128, 2, C], f8,
                            kind="ExternalInput").ap() for s in "hl"]
    w1_es = [nc.dram_tensor(f"w1{s}", [C, 4 * C], f8,
                            kind="ExternalInput").ap() for s in "hl"]
    if FP8W2:
        w2_es = [nc.dram_tensor(f"w2{s}", [4 * C, C], f8,
                                kind="ExternalInput").ap() for s in "hl"]
    else:
        w2_e = nc.dram_tensor("w2", [4 * C, C], bf16,
                              kind="ExternalInput").ap()
    b1r_e = nc.dram_tensor("b1r", [4 * C], f32, kind="ExternalInput").ap()
    b1s_e = nc.dram_tensor("b1s", [4 * C], f32, kind="ExternalInput").ap()
    # scales: col 0 qks=1/(SH^2 cq ck), 1 vsc=1/(SH cv), 2 wos=1/(SA cwo),
    #         3 rsc=SR/(SH c1), 4 w2s=1/(SR c2)
    sc_e = nc.dram_tensor("scales", [128, 8], f32, kind="ExternalInput").ap()
    if has_bqkv:
        bq_e = nc.dram_tensor("bq", [1, HPC * D], f32, kind="ExternalInput").ap()
        bk_e = nc.dram_tensor("bk", [1, HPC * D], f32, kind="ExternalInput").ap()
        bv_e = nc.dram_tensor("bv", [1, HPC * D], f32, kind="ExternalInput").ap()
    if has_bo:
        bo_e = nc.dram_tensor("bo", [C], f32, kind="ExternalInput").ap()
    if has_b2:
        b2_e = nc.dram_tensor("b2", [C], f32, kind="ExternalInput").ap()
    y_e = nc.dram_tensor("y", [TH, C], f32, kind="ExternalOutput").ap()

    cc_in = [nc.dram_tensor(f"cc_in{k}", [T // NCH, C], bf16)
             for k in range(NCH)]
    cc_out = [nc.dram_tensor(f"cc_out{k}", [T // NCH // 2, C], bf16)
              for k in range(NCH)]

    with tile.TileContext(nc) as tc, ExitStack() as ctx:
        psA = ctx.enter_context(tc.tile_pool(name="psA", bufs=2, space="PSUM"))
        psB = ctx.enter_context(tc.tile_pool(name="psB", bufs=2, space="PSUM"))
        consts = ctx.enter_context(tc.tile_pool(name="consts", bufs=1))
        smalls = ctx.enter_context(tc.tile_pool(name="smalls", bufs=2))
        qk_pool = ctx.enter_context(tc.tile_pool(name="qk", bufs=1))
        vpool = ctx.enter_context(tc.tile_pool(name="vp", bufs=1))
        attnp = ctx.enter_context(tc.tile_pool(name="attnp", bufs=1))
        fw = ctx.enter_context(tc.tile_pool(name="fw", bufs=1))
        x2p = ctx.enter_context(tc.tile_pool(name="x2p", bufs=1))
        h2p = ctx.enter_context(tc.tile_pool(name="h2p", bufs=1))

        # ---- constants ----
        ident_b = consts.tile([128, 128], bf16)
        make_identity(nc, ident_b)
        mb_b = consts.tile([128, 128], bf16)
        nc.gpsimd.memset(mb_b[:], 0.0)
        nc.gpsimd.affine_select(
            out=mb_b[:], in_=mb_b[:], compare_op=ALU.is_ge, fill=NEG,
            base=0, pattern=[[1, 128]], channel_multiplier=-1,
        )
        eps_t = consts.tile([128, 1], f32)
        nc.vector.memset(eps_t[:], EPS / (SH * SH))
        b1r_sb = consts.tile([128, 16], f32)
        b1s_sb = consts.tile([128, 16], f32)
        sc_sb = consts.tile([128, 8], f32)
        qks = sc_sb[:, 0:1]
        vsc = sc_sb[:, 1:2]
        wos = sc_sb[:, 2:3]
        rsc = sc_sb[:, 3:4]
        w2s = sc_sb[:, 4:5]
        if has_bqkv:
            ones_sb = consts.tile([1, 512], f32)
            nc.vector.memset(ones_sb[:], 1.0)
            bq_sb = consts.tile([1, HPC * D], f32)
            nc.sync.dma_start(bq_sb[:], bq_e)
            bk_sb = consts.tile([1, HPC * D], f32)
            nc.sync.dma_start(bk_sb[:], bk_e)
            bv_sb = consts.tile([1, HPC * D], f32)
            nc.sync.dma_start(bv_sb[:], bv_e)
        if has_bo:
            bo_sb = consts.tile([128, C], f32)
            bo_b = bo_e[None, :]
            bo_bc = bass.AP(
                tensor=bo_b.tensor, offset=bo_b.offset,
                ap=[[0, 128], bo_b.ap[1]],
            )
            nc.sync.dma_start(bo_sb[:], bo_bc)
        if has_b2:
            b2_sb = consts.tile([128, C], f32)
            b2_b = b2_e[None, :]
            b2_bc = bass.AP(
                tensor=b2_b.tensor, offset=b2_b.offset,
                ap=[[0, 128], b2_b.ap[1]],
            )
            nc.sync.dma_start(b2_sb[:], b2_bc)

        # persistent activations
        qT = qk_pool.tile([128, 2, T], bf16)  # [pair-head d, pair, t]
        kT = qk_pool.tile([128, 2, T], bf16)
        wq_sb = qk_pool.tile([128, 4, HPC * D], bf16)
        wk_sb = qk_pool.tile([128, 4, HPC * D], bf16)
        hT = qk_pool.tile([128, 4, T], bf16)
        v_sb = vpool.tile([128, NT, HPC * D], bf16)  # [s in tile, tile, hd]
        attn_hi = attnp.tile([128, 2, T], f8)  # [hd in pair, pair, t] * SA
        attn_lo = attnp.tile([128, 2, T], f8)

        # FFN weights + residual stream (DMA emitted inside phase A, after
        # the x loads, so the x tiles win the DMA queue)
        w1_sb = [fw.tile([128, 4, 4 * C], f8, name=f"w1_{s}")
                 for s in range(2)]
        if FP8W2:
            w2_sb = [fw.tile([128, 16, C], f8, name=f"w2_{s}")
                     for s in range(2)]
        else:
            w2_sb = fw.tile([128, 16, C], bf16)
        wo_sb = [fw.tile([128, 2, C], f8, name=f"wo_{s}")
                 for s in range(2)]
        xh_sb = fw.tile([128, 8, C], bf16)
        x2 = x2p.tile([128, 8, C], f32)
        h2T = [h2p.tile([128, 4, TH], f8, name=f"h2T_{s}")
               for s in range(2)]

        def layer_norm_tile(xm, hm, dve=False):
            """hm = SH * (xm - mean) * rsqrt(var + eps)."""
            stats = smalls.tile([128, 6], f32, tag="bnst")
            nc.vector.bn_stats(stats[:], xm)
            mv = smalls.tile([128, 2], f32, tag="bnag")
            nc.vector.bn_aggr(mv[:], stats[:])
            rstd = smalls.tile([128, 1], f32, tag="rstd")
            nc.scalar.activation(rstd[:], mv[:, 1:2], AF.Sqrt,
                                 bias=eps_t[:], scale=1.0 / (SH * SH))
            nc.vector.reciprocal(rstd[:], rstd[:])
            if dve:
                nc.vector.tensor_scalar(
                    hm, xm, mv[:, 0:1], rstd[:], ALU.subtract, ALU.mult
                )
                return
            nmr = smalls.tile([128, 1], f32, tag="nmr")
            nc.vector.tensor_scalar(
                nmr[:], mv[:, 0:1], rstd[:], -1.0, ALU.mult, ALU.mult
            )
            nc.scalar.activation(hm, xm, AF.Identity, bias=nmr[:],
                                 scale=rstd[:])

        # ================= Phase A: LN1 + QKV =================
        with ExitStack() as phaseA:
            wv_pool = phaseA.enter_context(tc.tile_pool(name="wvp", bufs=1))
            wv_sb = wv_pool.tile([128, 4, HPC * D], bf16)

            xpool = phaseA.enter_context(tc.tile_pool(name="xp", bufs=1))
            hpool = phaseA.enter_context(tc.tile_pool(name="hn", bufs=1))
            x_sb = xpool.tile([128, NT, C], bf16)
            x_r = x_e.rearrange("(n p) c -> p n c", p=128)
            for xc in range(16):
                nc.sync.dma_start(
                    x_sb[:, xc:xc + 1, :], x_r[:, xc:xc + 1, :],
                )
            nc.sync.dma_start(
                wq_sb[:], wq_e.rearrange("(o p) d -> p o d", p=128))
            nc.sync.dma_start(
                wk_sb[:], wk_e.rearrange("(o p) d -> p o d", p=128))
            nc.sync.dma_start(
                wv_sb[:], wv_e.rearrange("(o p) d -> p o d", p=128))
            for s in range(2):
                nc.sync.dma_start(
                    w1_sb[s][:], w1_es[s].rearrange("(o p) n -> p o n", p=128))
            for s in range(2):
                nc.sync.dma_start(wo_sb[s][:], wo_es[s])
            if FP8W2:
                for s in range(2):
                    nc.sync.dma_start(
                        w2_sb[s][:],
                        w2_es[s].rearrange("(o p) c -> p o c", p=128))
            else:
                nc.sync.dma_start(
                    w2_sb[:], w2_e.rearrange("(o p) c -> p o c", p=128))
            nc.sync.dma_start(xh_sb[:],
                              xh_e.rearrange("(n p) c -> p n c", p=128))
            nc.sync.dma_start(sc_sb[:], sc_e)
            nc.sync.dma_start(b1r_sb[:],
                              b1r_e.rearrange("(n p) -> p n", p=128))
            nc.sync.dma_start(b1s_sb[:],
                              b1s_e.rearrange("(n p) -> p n", p=128))

            def emit_qk(p, tbb, copy_dve=False, only=None):
                psl = slice(p * 128, (p + 1) * 128)
                pairs = []
                if only in (None, "q"):
                    qp = psA.tile([128, 1536], f32, tag="psA",
                                  name="qp")[:, :1024]
                    pairs.append((qp, wq_sb, "bq"))
                if only in (None, "k"):
                    kp = psA.tile([128, 1536], f32, tag="psA",
                                  name="kp")[:, :1024]
                    pairs.append((kp, wk_sb, "bk"))
                for dst, w_sb, b_sb in pairs:
                    for half in range(2):
                        t0 = tbb * 1024 + half * 512
                        sl = slice(half * 512, (half + 1) * 512)
                        for cc_ in range(4):
                            nc.tensor.matmul(
                                dst[:, sl],
                                lhsT=w_sb[:, cc_, psl],
                                rhs=hT[:, cc_, t0:t0 + 512],
                                start=(cc_ == 0),
                                stop=(cc_ == 3 and not has_bqkv),
                            )
                        if has_bqkv:
                            bsb = bq_sb if b_sb == "bq" else bk_sb
                            nc.tensor.matmul(
                                dst[:, sl],
                                lhsT=bsb[0:1, psl],
                                rhs=ones_sb[0:1, :],
                                start=False, stop=True, skip_group_check=True,
                            )
                tsl = slice(tbb * 1024, (tbb + 1) * 1024)
                if only in (None, "q"):
                    if copy_dve:
                        nc.vector.tensor_copy(qT[:, p, tsl], qp[:])
                    else:
                        nc.scalar.copy(qT[:, p, tsl], qp[:])
                if only in (None, "k"):
                    nc.vector.tensor_copy(kT[:, p, tsl], kp[:])

            hms = []
            for m in range(NT):
                hm = hpool.tile([128, C], bf16, tag=f"hm{m}", name=f"hm{m}")
                layer_norm_tile(x_sb[:, m, :], hm[:])
                hms.append(hm)
            for m in range(NT):
                hm = hms[m]
                tp = psB.tile([128, 4, 128], bf16, tag="psB", name="tp")
                for cc_ in range(4):
                    nc.tensor.transpose(
                        tp[:, cc_, :],
                        hm[:, cc_ * 128:(cc_ + 1) * 128],
                        ident_b[:],
                    )
                msl = slice(m * 128, (m + 1) * 128)
                nc.scalar.copy(hT[:, :, msl], tp[:])
            for m in range(NT):
                msl = slice(m * 128, (m + 1) * 128)
                # v for this token tile (all 4 heads along free axis)
                vp_ = psB.tile([128, 512], f32, tag="psB",
                               name="vp")[:, :HPC * D]
                for cc_ in range(4):
                    nc.tensor.matmul(
                        vp_,
                        lhsT=hT[:, cc_, msl],
                        rhs=wv_sb[:, cc_, :],
                        start=(cc_ == 0),
                        stop=(cc_ == 3 and not has_bqkv),
                    )
                if has_bqkv:
                    nc.tensor.matmul(
                        vp_,
                        lhsT=ones_sb[0:1, :128],
                        rhs=bv_sb[0:1, :],
                        start=False, stop=True, skip_group_check=True,
                    )
                nc.vector.tensor_copy(v_sb[:, m, :], vp_)
                if m == 7:
                    emit_qk(0, 0)
                if m == 15:
                    emit_qk(0, 1)
            _CACHE["_qk_defer"] = [
                lambda: emit_qk(1, 0, copy_dve=True, only="q"),
                lambda: emit_qk(1, 0, copy_dve=True, only="k"),
                lambda: emit_qk(1, 1, copy_dve=True, only="q"),
                lambda: emit_qk(1, 1, copy_dve=True, only="k"),
            ]

        # ================= Phase B: attention =================
        h2mp = ctx.enter_context(tc.tile_pool(name="h2mp", bufs=1))
        h2ms = {}
        pts = {}

        def cf_front(k):
            """residual + LN2 for chunk k (pt data already landed)."""
            for mm_ in range(2):
                m = 2 * k + mm_
                pt = pts[m]
                nc.vector.tensor_tensor(x2[:, m, :], xh_sb[:, m, :], pt[:],
                                        ALU.add)
                if has_bo:
                    nc.vector.tensor_tensor(
                        x2[:, m, :], x2[:, m, :], bo_sb[:], ALU.add
                    )
                hm = h2mp.tile([128, C], bf16, tag=f"h2m{m}", name=f"h2m{m}")
                layer_norm_tile(x2[:, m, :], hm[:], dve=True)
                h2ms[m] = hm

        def cf_tp_m(m):
            """transposes + fp8 hi/lo h2T for one token tile."""
            if True:
                hm = h2ms[m]
                tp = psB.tile([128, 4, 128], bf16, tag="psB", name="tp2")
                for cc_ in range(4):
                    nc.tensor.transpose(
                        tp[:, cc_, :],
                        hm[:, cc_ * 128:(cc_ + 1) * 128],
                        ident_b[:],
                    )
                msl = slice(m * 128, (m + 1) * 128)
                nc.scalar.copy(h2T[0][:, :, msl], tp[:])
                nc.vector.tensor_tensor(
                    h2T[1][:, :, msl], tp[:], h2T[0][:, :, msl], ALU.subtract
                )

        def cf_tp(k):
            cf_tp_m(2 * k)
            cf_tp_m(2 * k + 1)

        def emit_wo(k):
            for mm_ in range(NT // NCH):
                m = k * (NT // NCH) + mm_
                msl = slice(m * 128, (m + 1) * 128)
                if k == 3 and mm_ % 2 == 1:
                    pp = psA.tile([128, 1536], f32, tag="psA",
                                  name="pp")[:, :512]
                else:
                    pp = psB.tile([128, 512], f32, tag="psB", name="pp")
                nmm = 0
                for aa, wa in ((0, 0), (0, 1), (1, 0)):
                    nmm += 1
                    asrc = attn_hi if aa == 0 else attn_lo
                    nc.tensor.matmul(
                        pp[:],
                        lhsT=asrc[:, :, msl],
                        rhs=wo_sb[wa][:],
                        start=(nmm == 1), stop=(nmm == 3),
                        perf_mode=DR,
                    )
                pj = smalls.tile([128, 512], bf16, tag="pj")
                nc.vector.tensor_scalar(pj[:], pp[:], wos, None,
                                        ALU.mult)
                nc.sync.dma_start(
                    cc_in[k].ap()[mm_ * 128:(mm_ + 1) * 128, :], pj[:]
                )
            if sim:
                nc.sync.dma_start(cc_out[k].ap(),
                                  cc_in[k].ap()[:T // NCH // 2, :])
            else:
                nc.gpsimd.collective_compute(
                    "ReduceScatter",
                    ALU.add,
                    replica_groups=[[0, 1], [2, 3], [4, 5], [6, 7]],
                    ins=[cc_in[k].ap()],
                    outs=[cc_out[k].ap()],
                )
            for mm_ in range(2):
                m = 2 * k + mm_
                pt = h2mp.tile([128, 512], bf16, tag=f"pr{m}",
                               name=f"pr{m}")
                nc.sync.dma_start(
                    pt[:], cc_out[k].ap()[mm_ * 128:(mm_ + 1) * 128, :]
                )
                pts[m] = pt

        expp = ctx.enter_context(tc.tile_pool(name="expp", bufs=1))
        vsp = ctx.enter_context(tc.tile_pool(name="vsp", bufs=1))
        zp = ctx.enter_context(tc.tile_pool(name="zp", bufs=2))

        if True:
            pending = []  # delayed AV emission closures

            for h in range(HPC):
                p, u = h // 2, h % 2
                usl = slice(64 * u, 64 * u + 64)
                z = zp.tile([128, NT], f32, tag=f"z{h % 2}")
                zr = zp.tile([128, NT], f32, tag=f"zr{h % 2}")
                vs = vsp.tile([128, NT, D], bf16, tag=f"vs{h % 2}")
                exps = []

                def make_av(h, j, p, u, usl, exps, vs):
                    def av_emit():
                        av = psB.tile([128, 512], f32, tag="psB",
                                      name=f"av_{h}_{j}")
                        for ii in range(4 * j + 4):
                            off = 128 * ii - 512 * j
                            if off <= 0:
                                nc.tensor.matmul(
                                    av[usl, :],
                                    lhsT=vs[:, ii, :],
                                    rhs=exps[ii][:, -off:-off + 512],
                                    start=(ii == 0), stop=(ii == 4 * j + 3),
                                )
                            else:
                                nc.tensor.matmul(
                                    av[usl, off:],
                                    lhsT=vs[:, ii, :],
                                    rhs=exps[ii][:, 0:512 - off],
                                    start=False, stop=(ii == 4 * j + 3),
                                    skip_group_check=True,
                                )
                        jsl = slice(j * 512, (j + 1) * 512)
                        nc.vector.tensor_scalar(
                            attn_hi[usl, p, jsl], av[usl, :], SA, None,
                            ALU.mult,
                        )
                        nc.vector.scalar_tensor_tensor(
                            attn_lo[usl, p, jsl], av[usl, :], SA,
                            attn_hi[usl, p, jsl], ALU.mult, ALU.subtract,
                        )
                        if h == 3:
                            emit_wo(j)
                            if j >= 2:
                                cf_front(j - 2)
                    return av_emit

                for i in range(NT):
                    t0 = 128 * i
                    blk = 512 * (i // 4)
                    et = expp.tile([128, T - t0], bf16,
                                   tag=f"exp{h % 2 if i < 2 else 2}_{i}",
                                   name=f"exp_{h}_{i}")
                    exps.append(et)
                    ps = psA.tile([128, 1536], f32, tag="psA",
                                  name=f"sc_{h}_{i}")
                    nblocks = min(3, 4 - i // 4)
                    for sb in range(nblocks):
                        tstart = blk + 512 * sb
                        seg_lo = max(t0, tstart)
                        nc.tensor.matmul(
                            ps[:, seg_lo - blk:tstart + 512 - blk],
                            lhsT=kT[usl, p, i * 128:(i + 1) * 128],
                            rhs=qT[usl, p, seg_lo:tstart + 512],
                            start=True, stop=(sb > 0),
                        )
                        if sb == 0:
                            off = t0 - blk
                            nc.tensor.matmul(
                                ps[:, off:off + 128],
                                lhsT=ident_b[:],
                                rhs=mb_b[:],
                                start=False, stop=True,
                                skip_group_check=True,
                            )
                    hi1 = min(blk + 1536, T)
                    if hi1 >= T and i >= 8 and h < 3:
                        nc.scalar.activation(
                            et[:, 0:T - t0], ps[:, t0 - blk:T - blk],
                            AF.Exp, bias=0.0, scale=qks,
                        )
                        nc.vector.tensor_reduce(
                            z[:, i:i + 1], et[:, 0:T - t0],
                            mybir.AxisListType.X, ALU.add,
                        )
                    elif hi1 >= T:
                        nc.scalar.activation(
                            et[:, 0:T - t0], ps[:, t0 - blk:T - blk],
                            AF.Exp, bias=0.0, scale=qks,
                            accum_out=z[:, i:i + 1],
                        )
                    else:
                        ps2 = psB.tile([128, 512], f32, tag="psB",
                                       name=f"sc2_{h}_{i}")
                        nc.tensor.matmul(
                            ps2[:, 0:512],
                            lhsT=kT[usl, p, i * 128:(i + 1) * 128],
                            rhs=qT[usl, p, 1536:2048],
                            start=True, stop=True,
                        )
                        zpt = zp.tile([128, 1], f32, tag="zpart")
                        nc.scalar.activation(
                            et[:, 0:hi1 - t0], ps[:, t0 - blk:hi1 - blk],
                            AF.Exp, bias=0.0, scale=qks, accum_out=zpt[:],
                        )
                        nc.scalar.activation(
                            et[:, 1536 - t0:2048 - t0], ps2[:, 0:512],
                            AF.Exp, bias=0.0, scale=qks,
                        )
                        zpt2 = zp.tile([128, 1], f32, tag="zpart2")
                        nc.vector.tensor_reduce(
                            zpt2[:], et[:, 1536 - t0:2048 - t0],
                            mybir.AxisListType.X, ALU.add,
                        )
                        nc.vector.tensor_tensor(
                            z[:, i:i + 1], zpt[:], zpt2[:], ALU.add
                        )
                    nc.vector.reciprocal(zr[:, i:i + 1], z[:, i:i + 1])
                    nc.vector.tensor_scalar(
                        vs[:, i, :], v_sb[:, i, h * D:(h + 1) * D],
                        zr[:, i:i + 1], vsc, ALU.mult, ALU.mult,
                    )
                    if h == 0 and i in (3, 9) or (h, i) in ((1, 3), (1, 9)):
                        _CACHE["_qk_defer"].pop(0)()
                    if i % 4 == 1 and pending:
                        pending.pop(0)()
                    if i % 4 == 3:
                        pending.append(make_av(h, i // 4, p, u, usl, exps, vs))
            _CACHE["_pending_drain"] = pending

        # ================= Phase C: FFN =================
        relup = ctx.enter_context(tc.tile_pool(name="relup", bufs=1))
        if True:
            rdt = f8 if FP8W2 else bf16
            rlt = [relup.tile([128, 16, 512], rdt, tag=f"rl_{tb}",
                              name=f"rl_{tb}") for tb in range(2)]

            def emit_w2_tile(tb, mloc):
                m = tb * 4 + mloc
                if tb == 1:
                    f2 = psA.tile([128, 1536], f32, tag="psA",
                                  name="f2")[:, :512]
                else:
                    f2 = psB.tile([128, 512], f32, tag="psB", name="f2")
                if FP8W2:
                    nmm = 0
                    for wa in range(2):
                        for j in range(8):
                            nmm += 1
                            nc.tensor.matmul(
                                f2[:],
                                lhsT=rlt[tb][:, 2 * j:2 * j + 2,
                                             mloc * 128:(mloc + 1) * 128],
                                rhs=w2_sb[wa][:, 2 * j:2 * j + 2, :],
                                start=(nmm == 1), stop=(nmm == 16),
                                perf_mode=DR,
                            )
                else:
                    for j in range(16):
                        nc.tensor.matmul(
                            f2[:],
                            lhsT=rlt[tb][:, j, mloc * 128:(mloc + 1) * 128],
                            rhs=w2_sb[:, j, :],
                            start=(j == 0), stop=(j == 15),
                        )
                yt = smalls.tile([128, 512], f32, tag="yt")
                if FP8W2:
                    nc.vector.scalar_tensor_tensor(
                        yt[:], f2[:], w2s, x2[:, m, :], ALU.mult, ALU.add
                    )
                else:
                    nc.vector.tensor_tensor(yt[:], f2[:], x2[:, m, :],
                                            ALU.add)
                if has_b2:
                    nc.vector.tensor_tensor(
                        yt[:], yt[:], b2_sb[:], ALU.add
                    )
                nc.sync.dma_start(y_e[m * 128:(m + 1) * 128, :], yt[:])

            def emit_w1(tb, interleave=None):
                for nn in range(16):
                    if interleave is not None and nn >= 8 and nn % 2 == 0:
                        emit_w2_tile(interleave, (nn - 8) // 2)
                    fp = psA.tile([128, 1536], f32, tag="psA",
                                  name="fp")[:, :512]
                    nmm = 0
                    for wa, ha in ((0, 0), (0, 1), (1, 0)):
                        for j in range(2):
                            nmm += 1
                            nc.tensor.matmul(
                                fp[:],
                                lhsT=w1_sb[wa][:, 2 * j:2 * j + 2,
                                               nn * 128:(nn + 1) * 128],
                                rhs=h2T[ha][:, 2 * j:2 * j + 2,
                                            tb * 512:(tb + 1) * 512],
                                start=(nmm == 1), stop=(nmm == 6),
                                perf_mode=DR,
                            )
                    if FP8W2:
                        # rl = SR*relu_true in fp8
                        nc.scalar.activation(
                            rlt[tb][:, nn, :], fp[:], AF.Relu,
                            bias=b1s_sb[:, nn:nn + 1], scale=rsc,
                        )
                    elif nn % 2 == 0:
                        # rl = SH*c1*relu_true; W2 is pre-divided by SH*c1
                        nc.vector.tensor_scalar(
                            rlt[tb][:, nn, :], fp[:],
                            b1r_sb[:, nn:nn + 1], 0.0, ALU.add, ALU.max,
                        )
                    else:
                        nc.scalar.activation(
                            rlt[tb][:, nn, :], fp[:], AF.Relu,
                            bias=b1r_sb[:, nn:nn + 1], scale=1.0,
                        )

            def emit_w2(tb):
                for mloc in range(4):
                    emit_w2_tile(tb, mloc)

            drain = _CACHE.pop("_pending_drain")
            while drain:
                drain.pop(0)()      # AV(h3,3) + Wo(3) + cf_front(1)
            cf_tp(0)
            cf_tp(1)
            cf_front(2)
            cf_front(3)
            emit_w1(0)
            cf_tp(2)
            cf_tp(3)
            emit_w1(1, interleave=0)
            emit_w2(1)

    nc.compile()
    return nc
